# revision 1
# baseline (speedup 1.0000x reference)
"""Trainium2 Bass kernel for nn_Model_1245540515968 (gnn_message_passing).

Self-contained: kernel(**inputs) -> np.ndarray [128] per-structure energies.

Strategy (8 cores, graph/data parallel):
  - Shard by structure: core c owns structures [16c, 16c+16) and their atoms.
  - Edges assigned to the core owning their receiver; sorted by receiver and
    packed into 128-slot tiles spanning <= 8 receiver atoms each.
  - Algebraic restructure: with P[e,(s,r)] = onehot_species(sender)[s]*bess[e,r]
    (32 features) and sh_full[e,m] (16 real-spherical-harmonic cols), the
    per-atom invariant block A/Am collapses to
        U[(s,r), (atom,m)] = sum_e P[e,(s,r)] * sh_full[e,m] * mask[e,atom]
        Am_l[j, atom, m]   = W3_l[(s,r), j]^T U[(s,r), atom, m-block(l)]
    where W3_l = ((emb[s,:] (x) w_rad_l[r,:]) @ w_mix_l) * (2l+1)^-0.25 is a
    host-precomputed weight transform. U is built by one PE matmul per edge
    tile (contract over the 128 edge slots), i.e. gather/segment-sum become
    dense tensor-engine work. B = sum_lm Am^2, then B^2, species embedding,
    w_out contraction, and the per-structure segment-sum are small per-block
    matmul/vector ops.
  - Per-edge sender/receiver rows are host-pregathered from a packed
    [pos, onehot(species)] table (TRN2 SWDGE indirect DMA supports only one
    index per partition per instruction, making device-side per-edge gather
    instruction-bound).
"""
import os
import sys
from contextlib import ExitStack

import numpy as np

for _p in ("/opt/trn_rl_repo",):
    if _p not in sys.path and os.path.isdir(_p):
        sys.path.insert(0, _p)

import concourse.bass as bass
import concourse.tile as tile
from concourse import bacc, mybir
from concourse.bass import IndirectOffsetOnAxis
from concourse.bass_utils import run_bass_kernel_spmd

F32 = mybir.dt.float32
I32 = mybir.dt.int32
AX = mybir.AxisListType
OP = mybir.AluOpType
ACTF = mybir.ActivationFunctionType

N_ATOMS = 10000
N_EDGES = 200000
N_SPECIES = 4
N_RAD = 8
N_MAX = [8, 6, 4, 2]
K_MIX = 128
N_STRUCT = 128
CUTOFF = 5.0
N_CORES = 8
S_PER_CORE = N_STRUCT // N_CORES
P = 128
ASPAN = 8
TPB = 16  # tiles per atom-block (16 tiles * 8 slots = 128 atom slots)

# sh_full column order: [sh3 (7), l0-const (1), sh1 (3), sh2 (5)]
# chosen so (l3,l0) and (l1,l2) pair into contiguous 8-col m-groups.
M_OFF = {3: 0, 0: 7, 1: 8, 2: 11}
M_LEN = {0: 1, 1: 3, 2: 5, 3: 7}

C1 = 0.4886025119029199
C2A = 1.0925484305920792
C2B = 0.31539156525252005
C2C = 0.5462742152960396
C3A = 0.5900435899266435
C3B = 2.890611442640554
C3C = 0.4570457994644658
C3D = 0.3731763325901154
L0C = 0.28209479177387814


# ----------------------------------------------------------------------------
# Host preprocessing (index-derived structures + weight transforms)
# ----------------------------------------------------------------------------

def _preprocess(inputs):
    species = np.asarray(inputs['species'])
    senders = np.asarray(inputs['senders'])
    receivers = np.asarray(inputs['receivers'])
    batch_seg = np.asarray(inputs['batch_seg'])
    positions = np.asarray(inputs['positions'], dtype=np.float32)

    struct_starts = np.searchsorted(batch_seg, np.arange(N_STRUCT + 1))
    core_hi = struct_starts[(np.arange(N_CORES) + 1) * S_PER_CORE]

    edge_core = np.searchsorted(core_hi, receivers, side='right')
    cores = []
    for c in range(N_CORES):
        e_idx = np.nonzero(edge_core == c)[0]
        e_idx = e_idx[np.argsort(receivers[e_idx], kind='stable')]
        cores.append(dict(e_idx=e_idx, s_lo=c * S_PER_CORE))

    # tile packing
    for c in cores:
        rs = receivers[c['e_idx']]
        atoms, counts = np.unique(rs, return_counts=True)
        tiles = []
        cur, cur_e = [], 0
        ptr = 0
        for a, cnt in zip(atoms, counts):
            assert cnt <= P
            if len(cur) == ASPAN or cur_e + cnt > P:
                tiles.append(cur)
                cur, cur_e = [], 0
            cur.append((int(a), int(cnt), ptr))
            ptr += int(cnt)
            cur_e += int(cnt)
        if cur:
            tiles.append(cur)
        c['tiles'] = tiles
    nt_max = max(len(c['tiles']) for c in cores)
    NB = -(-nt_max // TPB)
    NT = NB * TPB

    for c in cores:
        send_idx = np.zeros((NT, P), np.int32)
        recv_idx = np.zeros((NT, P), np.int32)
        M = np.zeros((NT, P, ASPAN), np.float32)
        slot_atom = -np.ones((NB * P,), np.int64)
        e_idx = c['e_idx']
        for t, tile_atoms in enumerate(c['tiles']):
            s = 0
            for a_local, (a, cnt, ptr) in enumerate(tile_atoms):
                eds = e_idx[ptr:ptr + cnt]
                send_idx[t, s:s + cnt] = senders[eds]
                recv_idx[t, s:s + cnt] = receivers[eds]
                M[t, s:s + cnt, a_local] = 1.0
                slot_atom[t * ASPAN + a_local] = a
                s += cnt
        c['send_idx'] = send_idx
        c['recv_idx'] = recv_idx
        c['mmask'] = np.ascontiguousarray(
            M.transpose(1, 0, 2).reshape(P, NT * ASPAN))        # [128, NT*8]
        oh = np.zeros((N_SPECIES, NB * P), np.float32)
        S = np.zeros((NB, P, S_PER_CORE), np.float32)
        valid = slot_atom >= 0
        va = slot_atom[valid]
        oh[species[va], np.nonzero(valid)[0]] = 1.0
        S[np.nonzero(valid)[0] // P, np.nonzero(valid)[0] % P,
          batch_seg[va] - c['s_lo']] = 1.0
        c['slot_valid'] = valid
        c['slot_species'] = np.where(valid, np.where(valid, 0, 0) + (
            species[np.where(valid, slot_atom, 0)]), -1)
        c['sstr'] = np.ascontiguousarray(
            S.transpose(1, 0, 2).reshape(P, NB * S_PER_CORE))   # [128, NB*16]

    # weight transforms
    emb = np.asarray(inputs['emb'], np.float32)
    emb2 = np.asarray(inputs['emb2'], np.float32)
    w_out = np.asarray(inputs['w_out'], np.float32)
    scal = float(np.asarray(inputs['scaling'])[0])
    W3 = np.zeros((32, 4 * K_MIX), np.float32)
    for l in range(4):
        w_rad = np.asarray(inputs[f'w_rad{l}'], np.float32) * 0.5  # fcut 0.5 fold
        w_mix = np.asarray(inputs[f'w_mix{l}'], np.float32)
        n_l = N_MAX[l]
        W2 = np.einsum('sc,ri->sric', emb, w_rad).reshape(32, n_l * 16)
        w3 = (W2 @ w_mix) * (2 * l + 1) ** -0.25
        if l == 0:
            w3 = w3 * L0C  # l0 sh col is stored as constant L0C=1 -> fold here
        W3[:, l * K_MIX:(l + 1) * K_MIX] = w3
    E2s = (emb2 * w_out[None, :] * scal).astype(np.float32)     # [4, 128]
    cw = np.asarray(inputs['comp_weights'], np.float32)
    # fold composition term: per-structure sum of cw[species] (host weight-prep)
    cw_struct = np.zeros(N_STRUCT, np.float32)
    np.add.at(cw_struct, batch_seg, cw[species])

    gtab = np.concatenate([
        positions,
        (species[:, None] == np.arange(N_SPECIES)[None, :]).astype(np.float32),
        np.zeros((N_ATOMS, 1), np.float32)], axis=1)            # [N, 8]
    ones = np.ones((P, 1), np.float32)

    shared = dict(w3=W3, onesc=ones)
    in_maps = []
    for ci, c in enumerate(cores):
        m = dict(shared)
        m['einit'] = cw_struct[ci * S_PER_CORE:(ci + 1) * S_PER_CORE].reshape(
            S_PER_CORE, 1).copy()
        # host-side gather of per-slot sender/receiver rows (slot (t,p) ->
        # partition p, tile t). Device-side alternative (indirect DMA) costs
        # ~1 instruction per 128 indices on TRN2 SWDGE; host gather keeps the
        # Pool engine free.
        gs = gtab[c['send_idx']]            # [NT, 128, 8]
        gr = gtab[c['recv_idx']][:, :, :4]  # [NT, 128, 4]
        m['gsend'] = np.ascontiguousarray(
            gs.transpose(1, 0, 2).reshape(P, NT * 8))
        m['grecv'] = np.ascontiguousarray(
            gr.transpose(1, 0, 2).reshape(P, NT * 4))
        m['mmask'] = c['mmask']
        sp_slot = c['slot_species']
        e2full = np.where((sp_slot >= 0)[None, :],
                          E2s.T[:, np.clip(sp_slot, 0, 3)], 0.0).astype(np.float32)
        m['e2full'] = np.ascontiguousarray(e2full)          # [128 j, NB*128]
        m['sstr'] = c['sstr']
        in_maps.append(m)
    return in_maps, NT, NB


# ----------------------------------------------------------------------------
# Bass program
# ----------------------------------------------------------------------------

def _chunk_ranges(NB, n_chunks=2):
    """Split NB blocks into chunks (in tiles), block-aligned."""
    n_chunks = min(n_chunks, NB)
    base, rem = divmod(NB, n_chunks)
    out = []
    b0 = 0
    for i in range(n_chunks):
        nb = base + (1 if i < rem else 0)
        out.append((b0 * TPB, (b0 + nb) * TPB))
        b0 += nb
    return out


CFG = dict(nchunks=4, shexp_dve_mod=3, upool_bufs=2, u_tiles=8,
           epool_bufs=2, spool_bufs=2, small_psum=False, am_bufs=4,
           sq_dve_n=0, rec_eng='mix', geom_eng='pool', ucopy_eng='mix',
           pf_eng='pool', bess_eng='dve', ablate=())


def build_program(NT, NB, repeat=1, inputs_internal=False):
    cfg = CFG
    nc = bacc.Bacc("TRN2", target_bir_lowering=False, debug=False)
    kind = "Internal" if inputs_internal else "ExternalInput"

    gsend = nc.dram_tensor('gsend', [P, NT * 8], F32, kind=kind).ap()
    grecv = nc.dram_tensor('grecv', [P, NT * 4], F32, kind=kind).ap()
    mmask = nc.dram_tensor('mmask', [P, NT * ASPAN], F32, kind=kind).ap()
    w3 = nc.dram_tensor('w3', [32, 4 * K_MIX], F32, kind="ExternalInput").ap()
    einit = nc.dram_tensor('einit', [S_PER_CORE, 1], F32, kind="ExternalInput").ap()
    onesc = nc.dram_tensor('onesc', [P, 1], F32, kind="ExternalInput").ap()
    e2full = nc.dram_tensor('e2full', [P, NB * P], F32, kind=kind).ap()
    sstr = nc.dram_tensor('sstr', [P, NB * S_PER_CORE], F32, kind=kind).ap()
    eout = nc.dram_tensor('eout', [S_PER_CORE, 1], F32, kind="ExternalOutput").ap()

    with tile.TileContext(nc) as tc, ExitStack() as ctx:
        cpool = ctx.enter_context(tc.tile_pool(name="const", bufs=1))
        gpool = ctx.enter_context(tc.tile_pool(name="gath", bufs=1))
        tpool = ctx.enter_context(tc.tile_pool(name="temps", bufs=2))
        epool = ctx.enter_context(tc.tile_pool(name="shexp", bufs=cfg["epool_bufs"]))
        spool = ctx.enter_context(tc.tile_pool(name="sq", bufs=cfg["spool_bufs"]))
        upool = ctx.enter_context(tc.tile_pool(name="upsum", bufs=cfg["upool_bufs"], space="PSUM"))
        apool = ctx.enter_context(tc.tile_pool(name="ampsum", bufs=cfg["am_bufs"], space="PSUM"))
        if cfg["small_psum"]:
            smpool = ctx.enter_context(tc.tile_pool(name="smpsum", bufs=2, space="PSUM"))
        else:
            smpool = apool

        # ---- constants / per-core tables to SBUF ----
        w3_sb = cpool.tile([32, 4 * K_MIX], F32)
        nc.sync.dma_start(w3_sb[:], w3)
        ones_sb = cpool.tile([P, 1], F32)
        nc.sync.dma_start(ones_sb[:], onesc)
        e2_sb = cpool.tile([P, NB * P], F32)
        nc.sync.dma_start(e2_sb[:], e2full)
        sstr_sb = cpool.tile([P, NB * S_PER_CORE], F32)
        nc.sync.dma_start(sstr_sb[:], sstr)
        mm_sb = cpool.tile([P, NT, ASPAN], F32)
        nc.sync.dma_start(mm_sb[:], mmask.rearrange("p (t a) -> p t a", a=ASPAN))

        g_send = gpool.tile([P, NT, 8], F32)
        g_recv = gpool.tile([P, NT, 4], F32)
        sh = gpool.tile([P, NT, 16], F32)
        pf = gpool.tile([P, NT, 32], F32)
        u_sb = gpool.tile([32, TPB * P], F32)
        e_acc = cpool.tile([S_PER_CORE, 1], F32)
        nc.sync.dma_start(e_acc[:], einit)
        nc.gpsimd.memset(sh[:, :, M_OFF[0]:M_OFF[0] + 1], 1.0)
        bias_eps = cpool.tile([P, 1], F32)
        nc.gpsimd.memset(bias_eps[:], 1e-12)
        bias_hpi = cpool.tile([P, 1], F32)
        nc.gpsimd.memset(bias_hpi[:], float(np.pi / 2))
        bias_npi = cpool.tile([P, 1], F32)
        nc.gpsimd.memset(bias_npi[:], float(-np.pi))

        chunks = _chunk_ranges(NB, cfg["nchunks"])

        # ---- per-chunk loads + geometry ----
        for _rep in range(repeat):
          for (t0, t1) in chunks:
              T = t1 - t0
              for (d0, d1) in _chunk_ranges(NB, 4):
                  if d0 < t0 or d0 >= t1:
                      continue
                  nc.sync.dma_start(
                      g_send[:, d0:d1, :],
                      gsend.rearrange("p (t c) -> p t c", c=8)[:, d0:d1, :])
                  nc.sync.dma_start(
                      g_recv[:, d0:d1, :],
                      grecv.rearrange("p (t c) -> p t c", c=4)[:, d0:d1, :])

              GEO = nc.gpsimd if cfg['geom_eng'] == 'pool' else nc.vector
              rvec = tpool.tile([P, T, 3], F32, tag="rvec")
              nc.vector.tensor_tensor(rvec[:], g_recv[:, t0:t1, 0:3],
                                      g_send[:, t0:t1, 0:3], OP.subtract)
              sq3 = tpool.tile([P, T, 3], F32, tag="sq3")
              GEO.tensor_tensor(sq3[:], rvec[:], rvec[:], OP.mult)
              r2 = tpool.tile([P, T], F32, tag="r2")
              nc.vector.tensor_reduce(r2[:], sq3[:], axis=AX.X, op=OP.add)
              r = tpool.tile([P, T], F32, tag="r")
              nc.scalar.activation(r[:], r2[:], ACTF.Sqrt, bias=bias_eps[:])
              rinv = tpool.tile([P, T], F32, tag="rinv")
              nc.vector.reciprocal(rinv[:], r[:])
              xr = tpool.tile([P, T], F32, tag="xr")
              nc.vector.tensor_scalar(xr[:], r[:], 1.0 / CUTOFF, 1.0, OP.mult, OP.min)
              xrp = tpool.tile([P, T], F32, tag="xrp")
              GEO.tensor_scalar(xrp[:], xr[:], 1e-3, None, OP.add)
              xrinv = tpool.tile([P, T], F32, tag="xrinv")
              nc.vector.reciprocal(xrinv[:], xrp[:])
              u = tpool.tile([P, T, 3], F32, tag="u")
              nc.vector.tensor_tensor(
                  u[:], rvec[:], rinv[:].unsqueeze(2).broadcast_to([P, T, 3]), OP.mult)
              fc = tpool.tile([P, T], F32, tag="fc")
              nc.scalar.activation(fc[:], xr[:], ACTF.Sin, bias=bias_hpi[:], scale=float(-np.pi))
              # sin(n*pi*xr) via Chebyshev recurrence: s_{n+1} = 2*cos(t)*s_n - s_{n-1}
              sin_t = tpool.tile([P, T, N_RAD], F32, tag="sin_t")
              nc.scalar.activation(sin_t[:, :, 0:1],
                                   xr[:].unsqueeze(2), ACTF.Sin, scale=float(np.pi))
              cc = tpool.tile([P, T, 1], F32, tag="cc")
              _R = {'mix': None, 'dve': nc.vector, 'pool': nc.gpsimd}[cfg['rec_eng']]
              nc.gpsimd.tensor_scalar(cc[:], fc[:].unsqueeze(2), 2.0, None, OP.mult)
              nc.vector.tensor_tensor(sin_t[:, :, 1:2], cc[:], sin_t[:, :, 0:1], OP.mult)
              stmp = tpool.tile([P, T, 1], F32, tag="stmp")
              for n in range(3, N_RAD + 1):
                  eng = _R or (nc.gpsimd if n % 2 else nc.vector)
                  eng2 = _R or (nc.vector if n % 2 else nc.gpsimd)
                  eng.tensor_tensor(stmp[:], cc[:], sin_t[:, :, n - 2:n - 1], OP.mult)
                  eng2.tensor_tensor(sin_t[:, :, n - 1:n], stmp[:],
                                     sin_t[:, :, n - 3:n - 2], OP.subtract)
              fc1 = tpool.tile([P, T], F32, tag="fc1")
              GEO.tensor_scalar(fc1[:], fc[:], 1.0, None, OP.add)
              wfac = tpool.tile([P, T], F32, tag="wfac")
              nc.vector.tensor_tensor(wfac[:], fc1[:], xrinv[:], OP.mult)
              bess = tpool.tile([P, T, N_RAD], F32, tag="bess")
              _BE = nc.gpsimd if cfg['bess_eng'] == 'pool' else nc.vector
              _BE.tensor_tensor(
                  bess[:], sin_t[:], wfac[:].unsqueeze(2).broadcast_to([P, T, N_RAD]),
                  OP.mult)
              # P features: onehot (x) bess -> [P, T, 4, 8]
              _PE2 = nc.gpsimd if cfg['pf_eng'] == 'pool' else nc.vector
              _PE2.tensor_tensor(
                  pf[:, t0:t1, :].rearrange("p t (s r) -> p t s r", s=4),
                  g_send[:, t0:t1, 3:7].unsqueeze(3).broadcast_to([P, T, 4, N_RAD]),
                  bess[:].unsqueeze(2).broadcast_to([P, T, 4, N_RAD]), OP.mult)

              # spherical harmonics into sh[:, t0:t1, :]
              x = u[:, :, 0:1]
              y = u[:, :, 1:2]
              z = u[:, :, 2:3]
              shc = sh[:, t0:t1, :]
              # l1: cols M_OFF[1]+(y,z,x)
              nc.vector.tensor_scalar(shc[:, :, M_OFF[1]:M_OFF[1] + 2],
                                      u[:, :, 1:3], C1, None, OP.mult)
              nc.vector.tensor_scalar(shc[:, :, M_OFF[1] + 2:M_OFF[1] + 3],
                                      x, C1, None, OP.mult)
              pr2 = tpool.tile([P, T, 2], F32, tag="pr2")  # (xy, yz)
              GEO.tensor_tensor(pr2[:], u[:, :, 0:2], u[:, :, 1:3], OP.mult)
              przx = tpool.tile([P, T, 1], F32, tag="przx")  # xz
              GEO.tensor_tensor(przx[:], z, x, OP.mult)
              u2 = tpool.tile([P, T, 3], F32, tag="u2")
              GEO.tensor_tensor(u2[:], u[:], u[:], OP.mult)
              x2 = u2[:, :, 0:1]
              y2 = u2[:, :, 1:2]
              z2 = u2[:, :, 2:3]
              # l2 block at M_OFF[2]: [C2A*xy, C2A*yz, C2B*(3z2-1), C2A*xz, C2C*(x2-y2)]
              o2 = M_OFF[2]
              nc.vector.tensor_scalar(shc[:, :, o2:o2 + 2], pr2[:], C2A, None, OP.mult)
              nc.vector.tensor_scalar(shc[:, :, o2 + 2:o2 + 3], z2,
                                      3.0 * C2B, C2B, OP.mult, OP.subtract)
              nc.vector.tensor_scalar(shc[:, :, o2 + 3:o2 + 4], przx[:], C2A, None, OP.mult)
              xmy = tpool.tile([P, T, 1], F32, tag="xmy")
              GEO.tensor_tensor(xmy[:], x2, y2, OP.subtract)
              nc.vector.tensor_scalar(shc[:, :, o2 + 4:o2 + 5], xmy[:], C2C, None, OP.mult)
              # l3 block at M_OFF[3]=0:
              # [C3A*y*(3x2-y2), C3B*xy*z, C3C*y*(5z2-1), C3D*z*(5z2-3),
              #  C3C*x*(5z2-1), C3B2*z*(x2-y2), C3A*x*(x2-3y2)]
              s3a = tpool.tile([P, T, 1], F32, tag="s3a")
              GEO.tensor_scalar(s3a[:], x2, 3.0 * C3A, None, OP.mult)
              s3c = tpool.tile([P, T, 1], F32, tag="s3c")
              GEO.tensor_scalar(s3c[:], y2, C3A, None, OP.mult)
              s3b = tpool.tile([P, T, 1], F32, tag="s3b")
              GEO.tensor_tensor(s3b[:], s3a[:], s3c[:], OP.subtract)
              nc.vector.tensor_tensor(shc[:, :, 0:1], s3b[:], y, OP.mult)
              zc = tpool.tile([P, T, 1], F32, tag="zc")
              GEO.tensor_scalar(zc[:], z, C3B, None, OP.mult)
              nc.vector.tensor_tensor(shc[:, :, 1:2], pr2[:, :, 0:1], zc[:], OP.mult)
              t511 = tpool.tile([P, T, 1], F32, tag="t511")
              GEO.tensor_scalar(t511[:], z2, 5.0 * C3C, C3C, OP.mult, OP.subtract)
              nc.vector.tensor_tensor(shc[:, :, 2:3], y, t511[:], OP.mult)
              t533 = tpool.tile([P, T, 1], F32, tag="t533")
              GEO.tensor_scalar(t533[:], z2, 5.0 * C3D, 3.0 * C3D, OP.mult, OP.subtract)
              nc.vector.tensor_tensor(shc[:, :, 3:4], z, t533[:], OP.mult)
              nc.vector.tensor_tensor(shc[:, :, 4:5], x, t511[:], OP.mult)
              zc2 = tpool.tile([P, T, 1], F32, tag="zc2")
              GEO.tensor_scalar(zc2[:], z, 1.445305721320277, None, OP.mult)
              nc.vector.tensor_tensor(shc[:, :, 5:6], xmy[:], zc2[:], OP.mult)
              s4a = tpool.tile([P, T, 1], F32, tag="s4a")
              GEO.tensor_scalar(s4a[:], x2, C3A, None, OP.mult)
              s4b = tpool.tile([P, T, 1], F32, tag="s4b")
              GEO.tensor_scalar(s4b[:], y2, 3.0 * C3A, None, OP.mult)
              s4c = tpool.tile([P, T, 1], F32, tag="s4c")
              GEO.tensor_tensor(s4c[:], s4a[:], s4b[:], OP.subtract)
              nc.vector.tensor_tensor(shc[:, :, 6:7], s4c[:], x, OP.mult)

              # ---- per-block scatter + phase 2 for blocks in this chunk ----
              for b in range(t0 // TPB, t1 // TPB):
                  if 'blocks' in cfg['ablate']:
                      continue
                  sh_exp = epool.tile([P, TPB, ASPAN, 16], F32, tag="shexp")
                  eng = nc.vector if (b % cfg["shexp_dve_mod"] == 0) else nc.gpsimd
                  eng.tensor_tensor(
                      sh_exp[:],
                      sh[:, b * TPB:(b + 1) * TPB, :].unsqueeze(2)
                        .broadcast_to([P, TPB, ASPAN, 16]),
                      mm_sb[:, b * TPB:(b + 1) * TPB, :].unsqueeze(3)
                        .broadcast_to([P, TPB, ASPAN, 16]),
                      OP.mult)
                  UT = cfg["u_tiles"]
                  if 'scatter' in cfg['ablate']:
                      continue
                  for hb in range(TPB // UT):
                      u_ps = upool.tile([32, UT * P], F32, tag="ups", space="PSUM")
                      for tl in range(UT):
                          t = b * TPB + hb * UT + tl
                          nc.tensor.matmul(
                              u_ps[:, tl * P:(tl + 1) * P],
                              lhsT=pf[:, t, :],
                              rhs=sh_exp[:, hb * UT + tl, :, :].rearrange(
                                  "p a m -> p (a m)"),
                              start=True, stop=True)
                      _uc = cfg['ucopy_eng']
                      use_act = (_uc == 'act') or (_uc == 'mix' and hb % 2 == 0)
                      if use_act:
                          nc.scalar.copy(u_sb[:, hb * UT * P:(hb + 1) * UT * P], u_ps[:])
                      else:
                          nc.vector.tensor_copy(u_sb[:, hb * UT * P:(hb + 1) * UT * P], u_ps[:])

                  # phase 2: Am matmuls; psum col layout (ta, m8) interleaved
                  uv = u_sb[:].rearrange("q (ta m) -> q ta m", m=16)
                  sq = spool.tile([P, P, 16], F32, tag="sq")
                  if 'p2' in cfg['ablate']:
                      continue
                  for pair_i, (la, lb) in enumerate(((3, 0), (1, 2))):
                      moff_a, mlen_a = M_OFF[la], M_LEN[la]
                      moff_b, mlen_b = M_OFF[lb], M_LEN[lb]
                      ca = 64 * mlen_a
                      cb = 64 * mlen_b
                      for hh in range(2):
                          am = apool.tile([P, 512], F32, tag="am", space="PSUM")
                          ta0 = hh * 64
                          nc.tensor.matmul(
                              am[:, 0:ca],
                              lhsT=w3_sb[:, la * K_MIX:(la + 1) * K_MIX],
                              rhs=uv[:, ta0:ta0 + 64, moff_a:moff_a + mlen_a],
                              start=True, stop=True)
                          nc.tensor.matmul(
                              am[:, ca:ca + cb],
                              lhsT=w3_sb[:, lb * K_MIX:(lb + 1) * K_MIX],
                              rhs=uv[:, ta0:ta0 + 64, moff_b:moff_b + mlen_b],
                              start=True, stop=True)
                          # square PSUM -> sq SBUF at (ta, m) grid
                          sqi = pair_i * 4 + hh * 2
                          for (off, cols, moff, mlen) in (
                                  (0, ca, moff_a, mlen_a), (ca, cb, moff_b, mlen_b)):
                              if sqi % 8 < cfg['sq_dve_n']:
                                  nc.vector.tensor_tensor(
                                      sq[:, ta0:ta0 + 64, moff:moff + mlen],
                                      am[:, off:off + cols].rearrange(
                                          "p (ta m) -> p ta m", m=mlen),
                                      am[:, off:off + cols].rearrange(
                                          "p (ta m) -> p ta m", m=mlen), OP.mult)
                              else:
                                  nc.scalar.activation(
                                      sq[:, ta0:ta0 + 64, moff:moff + mlen],
                                      am[:, off:off + cols].rearrange(
                                          "p (ta m) -> p ta m", m=mlen),
                                      ACTF.Square)
                              sqi += 1
                  if 'sqred' in cfg['ablate']:
                      continue
                  B = spool.tile([P, P], F32, tag="B")
                  nc.vector.tensor_reduce(B[:], sq[:], axis=AX.X, op=OP.add)
                  B4 = spool.tile([P, P], F32, tag="B4")
                  nc.gpsimd.tensor_tensor(B4[:], B[:], B[:], OP.mult)
                  H = spool.tile([P, P], F32, tag="H")
                  nc.gpsimd.tensor_tensor(H[:], B4[:],
                                          e2_sb[:, b * P:(b + 1) * P], OP.mult)
                  at_ps = smpool.tile([P, 1], F32, tag="sm" if cfg["small_psum"] else "am", space="PSUM")
                  nc.tensor.matmul(at_ps[:], lhsT=H[:], rhs=ones_sb[:],
                                   start=True, stop=True)
                  at_sb = spool.tile([P, 1], F32, tag="at")
                  nc.scalar.copy(at_sb[:], at_ps[:])
                  eb_ps = smpool.tile([S_PER_CORE, 1], F32, tag="sm" if cfg["small_psum"] else "am", space="PSUM")
                  nc.tensor.matmul(
                      eb_ps[:], lhsT=sstr_sb[:, b * S_PER_CORE:(b + 1) * S_PER_CORE],
                      rhs=at_sb[:], start=True, stop=True)
                  nc.vector.tensor_tensor(e_acc[:], e_acc[:], eb_ps[:], OP.add)

        nc.sync.dma_start(eout, e_acc[:])

    nc.compile()
    return nc


_CACHE = {}


def _get_program(NT, NB):
    key = (NT, NB)
    if key not in _CACHE:
        _CACHE[key] = build_program(NT, NB)
    return _CACHE[key]


def run(inputs, trace=False, **kwargs):
    in_maps, NT, NB = _preprocess(inputs)
    nc = _get_program(NT, NB)
    res = run_bass_kernel_spmd(nc, in_maps, core_ids=list(range(N_CORES)),
                               trace=trace, **kwargs)
    out = np.concatenate([res.results[c]['eout'][:, 0] for c in range(N_CORES)])
    return out.astype(np.float32), res


def kernel(**inputs):
    out, _ = run(inputs)
    return out



# revision 6
# speedup vs baseline: 1.0688x; 1.0688x over previous
"""Trainium2 Bass kernel for nn_Model_1245540515968 (gnn_message_passing), v2.

Self-contained: kernel(**inputs) -> np.ndarray [128] per-structure energies.

Strategy (8 cores, graph/data parallel, same algebra as v1 but restructured):
  - Shard by structure: core c owns structures [16c,16c+16) and their atoms.
  - Atoms packed 6-per-tile (128 edge slots); ASPAN=6, TPB=20 tiles/block
    -> 120 atom slots per block, NB blocks.
  - fp16 matmul operands everywhere (PE 1 cyc/row vs fp32's 4).
  - Host-expanded mask mm_exp [P,NT,6,16] fp16 so sh_exp = STT(sh, mm) hits
    the DVE 4x_2p mode (0.26 ns/elem).
  - All elementwise via scalar_tensor_tensor / tensor_scalar (TensorScalarPtr
    -> 2x_2p/4x_2p modes) instead of tensor_tensor (2x_1p only).
  - Newton fast-inverse-sqrt on DVE so Act only needs {sin, square, copy}
    (single act table: trig_and_small; no 1283ns table switches).
  - Phase-1 U matmuls write 4 partition-groups of PSUM (32 rows each), so
    the PSUM->SBUF U copy is [128,480] (484 free elems) not [32,1920].
  - Phase 2: Am = w3_l^T U per (l, g-pair); Act Square PSUM->SBUF fp16;
    DVE pairwise-add tree over m (4x mode); B4/H fp16; per-block energy via
    two small matmuls; fp32 accumulation of per-structure energies.
"""
import os
import sys
from contextlib import ExitStack

import numpy as np

for _p in ("/opt/trn_rl_repo",):
    if _p not in sys.path and os.path.isdir(_p):
        sys.path.insert(0, _p)

import concourse.bass as bass
import concourse.tile as tile
from concourse import bacc, mybir
from concourse.bass_utils import run_bass_kernel_spmd

F32 = mybir.dt.float32
F16 = mybir.dt.float16
I32 = mybir.dt.int32
AX = mybir.AxisListType
OP = mybir.AluOpType
ACTF = mybir.ActivationFunctionType

N_ATOMS = 10000
N_EDGES = 200000
N_SPECIES = 4
N_RAD = 8
N_MAX = [8, 6, 4, 2]
K_MIX = 128
N_STRUCT = 128
CUTOFF = 5.0
N_CORES = 8
S_PER_CORE = N_STRUCT // N_CORES
P = 128
ASPAN = 6           # atoms (lanes) per tile
TPB = 20            # tiles per block
TA = ASPAN * TPB    # atom slots per block = 120
NGRP = 4            # psum partition groups (tl % 4)
CPG = TPB // NGRP   # tiles per group = 5

# sh_full column order: [sh3 (7), l0-const (1), sh1 (3), sh2 (5)]
M_OFF = {3: 0, 0: 7, 1: 8, 2: 11}
M_LEN = {0: 1, 1: 3, 2: 5, 3: 7}

C1 = 0.4886025119029199
C2A = 1.0925484305920792
C2B = 0.31539156525252005
C2C = 0.5462742152960396
C3A = 0.5900435899266435
C3B = 2.890611442640554
C3B2 = 1.445305721320277
C3C = 0.4570457994644658
C3D = 0.3731763325901154
L0C = 0.28209479177387814
MAGIC = 0x5F3759DF


# ----------------------------------------------------------------------------
# Host preprocessing
# ----------------------------------------------------------------------------

def _pack_tiles(atom_ids, degs, nbins):
    """Worst-fit decreasing into a fixed number of bins, each <=ASPAN atoms
    and <=P edge slots. Returns list of tiles (atom-id lists) or None."""
    import heapq
    order = np.argsort(-degs, kind='stable')
    heap = [(-P, j) for j in range(nbins)]
    heapq.heapify(heap)
    used = [0] * nbins
    tiles = [[] for _ in range(nbins)]
    for i in order:
        a, d = int(atom_ids[i]), int(degs[i])
        tmp = []
        placed = False
        while heap:
            negfree, j = heapq.heappop(heap)
            if len(tiles[j]) < ASPAN and used[j] + d <= P:
                used[j] += d
                tiles[j].append(a)
                if len(tiles[j]) < ASPAN:
                    heapq.heappush(heap, (-(P - used[j]), j))
                placed = True
                break
            tmp.append((negfree, j))
            if -negfree < d:
                break
        for it in tmp:
            heapq.heappush(heap, it)
        if not placed:
            return None
    return tiles


def _preprocess(inputs):
    species = np.asarray(inputs['species'])
    senders = np.asarray(inputs['senders'])
    receivers = np.asarray(inputs['receivers'])
    batch_seg = np.asarray(inputs['batch_seg'])
    positions = np.asarray(inputs['positions'], dtype=np.float32)

    struct_starts = np.searchsorted(batch_seg, np.arange(N_STRUCT + 1))
    core_hi = struct_starts[(np.arange(N_CORES) + 1) * S_PER_CORE]
    core_lo = np.concatenate([[0], core_hi[:-1]])

    # receiver-sorted edge index
    r_order = np.argsort(receivers, kind='stable')
    r_sorted = receivers[r_order]
    deg = np.bincount(receivers, minlength=N_ATOMS)
    estart = np.concatenate([[0], np.cumsum(deg)])

    amax = max(int(core_hi[c] - core_lo[c]) for c in range(N_CORES))
    NT = -(-(-(-amax // ASPAN)) // TPB) * TPB
    while True:
        cores = []
        for c in range(N_CORES):
            a_lo, a_hi = int(core_lo[c]), int(core_hi[c])
            aids = np.arange(a_lo, a_hi)
            tiles = _pack_tiles(aids, deg[a_lo:a_hi], NT)
            if tiles is None:
                cores = None
                break
            cores.append(dict(a_lo=a_lo, a_hi=a_hi, tiles=tiles))
        if cores is not None:
            break
        NT += TPB
    NB = NT // TPB

    # weight transforms
    emb = np.asarray(inputs['emb'], np.float32)
    emb2 = np.asarray(inputs['emb2'], np.float32)
    w_out = np.asarray(inputs['w_out'], np.float32)
    scal = float(np.asarray(inputs['scaling'])[0])
    W3 = np.zeros((32, 4 * K_MIX), np.float32)
    for l in range(4):
        w_rad = np.asarray(inputs[f'w_rad{l}'], np.float32) * 0.5  # fcut 0.5
        w_mix = np.asarray(inputs[f'w_mix{l}'], np.float32)
        n_l = N_MAX[l]
        W2 = np.einsum('sc,ri->sric', emb, w_rad).reshape(32, n_l * 16)
        w3 = (W2 @ w_mix) * (2 * l + 1) ** -0.25
        if l == 0:
            w3 = w3 * L0C
        W3[:, l * K_MIX:(l + 1) * K_MIX] = w3
    E2s = (emb2 * w_out[None, :] * scal).astype(np.float32)     # [4, 128]
    cw = np.asarray(inputs['comp_weights'], np.float32)
    cw_struct = np.zeros(N_STRUCT, np.float32)
    np.add.at(cw_struct, batch_seg, cw[species])

    oh_tab = (species[:, None] == np.arange(N_SPECIES)[None, :]).astype(np.float16)

    w3q = np.zeros((P, 16 * K_MIX), np.float32)
    for l in range(4):
        for g in range(NGRP):
            w3q[32 * g:32 * (g + 1), (l * NGRP + g) * K_MIX:
                (l * NGRP + g + 1) * K_MIX] = W3[:, l * K_MIX:(l + 1) * K_MIX]
    shared = dict(w3=np.ascontiguousarray(w3q).astype(np.float16))
    in_maps = []
    for ci, c in enumerate(cores):
        g6 = np.zeros((NT, P, 6), np.float32)
        goh = np.zeros((NT, P, 4), np.float16)
        mm = np.zeros((NT, P, ASPAN), np.float16)
        slot_atom = -np.ones((NB, TA), np.int64)   # [block, ta] -> atom
        for t, tile_atoms in enumerate(c['tiles']):
            b, tl = t // TPB, t % TPB
            g, cc_ = tl % NGRP, tl // NGRP
            s = 0
            for a_local, a in enumerate(tile_atoms):
                d = int(deg[a])
                eds = r_order[estart[a]:estart[a] + d]
                g6[t, s:s + d, 0:3] = positions[senders[eds]]
                g6[t, s:s + d, 3:6] = positions[receivers[eds]]
                goh[t, s:s + d, :] = oh_tab[senders[eds]]
                mm[t, s:s + d, a_local] = 1.0
                ta = g * (CPG * ASPAN) + cc_ * ASPAN + a_local
                slot_atom[b, ta] = a
                s += d
        mm_exp = np.broadcast_to(mm[:, :, :, None], (NT, P, ASPAN, 16))
        m = dict(shared)
        m['g6'] = np.ascontiguousarray(g6.transpose(1, 0, 2).reshape(P, NT * 6))
        m['goh'] = np.ascontiguousarray(goh.transpose(1, 0, 2).reshape(P, NT * 4))
        m['mmx'] = np.ascontiguousarray(
            mm_exp.transpose(1, 0, 2, 3).reshape(P, NT * ASPAN * 16))
        sa = slot_atom.reshape(-1)
        valid = sa >= 0
        sp_slot = np.where(valid, species[np.clip(sa, 0, None)], 0)
        e2full = np.where(valid[None, :], E2s.T[:, sp_slot], 0.0)
        m['e2full'] = np.ascontiguousarray(e2full).astype(np.float16)  # [128, NB*TA]
        S = np.zeros((NB, TA, S_PER_CORE), np.float16)
        bidx = np.nonzero(valid)[0]
        S[bidx // TA, bidx % TA,
          batch_seg[sa[valid]] - ci * S_PER_CORE] = 1.0
        sfull = np.zeros((P, NB * S_PER_CORE), np.float16)
        sfull[:TA, :] = S.transpose(1, 0, 2).reshape(TA, NB * S_PER_CORE)
        m['sstr'] = np.ascontiguousarray(sfull)
        m['einit'] = cw_struct[ci * S_PER_CORE:(ci + 1) * S_PER_CORE].reshape(
            S_PER_CORE, 1).copy()
        m['onesc'] = np.ones((P, 1), np.float16)
        m['zed'] = np.zeros((P, NT * 128), np.float16)
        in_maps.append(m)
    return in_maps, NT, NB


# ----------------------------------------------------------------------------
# Bass program
# ----------------------------------------------------------------------------

def _chunks(NB, n):
    n = min(n, NB)
    base, rem = divmod(NB, n)
    out, b0 = [], 0
    for i in range(n):
        nb = base + (1 if i < rem else 0)
        out.append((b0, b0 + nb))
        b0 += nb
    return out


def _grow_chunks(NB, first=1, mult=2):
    """Geometrically growing chunk sizes: [1, 2, 4, 4, ...] summing to NB."""
    out, b0, sz = [], 0, first
    while b0 < NB:
        nb = min(sz, NB - b0)
        out.append((b0, b0 + nb))
        b0 += nb
        sz = min(sz * mult, 4)
    return out


CFG = dict(nchunks=2, ndma=4, sq_dve=(), geo_pool=('sq3', 'u2', 'prods'),
           pf_eng='pool', b4h_eng='pool', shexp_pool_mod=5)


def build_program(NT, NB, repeat=1):
    cfg = CFG
    nc = bacc.Bacc("TRN2", target_bir_lowering=False, debug=False)

    g6d = nc.dram_tensor('g6', [P, NT * 6], F32, kind="ExternalInput").ap()
    gohd = nc.dram_tensor('goh', [P, NT * 4], F16, kind="ExternalInput").ap()
    mmxd = nc.dram_tensor('mmx', [P, NT * ASPAN * 16], F16, kind="ExternalInput").ap()
    w3d = nc.dram_tensor('w3', [P, 16 * K_MIX], F16, kind="ExternalInput").ap()
    e2d = nc.dram_tensor('e2full', [P, NB * TA], F16, kind="ExternalInput").ap()
    sstrd = nc.dram_tensor('sstr', [P, NB * S_PER_CORE], F16, kind="ExternalInput").ap()
    einitd = nc.dram_tensor('einit', [S_PER_CORE, 1], F32, kind="ExternalInput").ap()
    onesd = nc.dram_tensor('onesc', [P, 1], F16, kind="ExternalInput").ap()
    zd = nc.dram_tensor('zed', [P, NT * 128], F16, kind="ExternalInput").ap()
    eout = nc.dram_tensor('eout', [S_PER_CORE, 1], F32, kind="ExternalOutput").ap()

    V = None  # set below per-engine helpers

    with tile.TileContext(nc) as tc, ExitStack() as ctx:
        cpool = ctx.enter_context(tc.tile_pool(name="const", bufs=1))
        gpool = ctx.enter_context(tc.tile_pool(name="gath", bufs=1))
        tpool = ctx.enter_context(tc.tile_pool(name="temps", bufs=2))
        epool = ctx.enter_context(tc.tile_pool(name="shexp", bufs=cfg['epool_bufs']))
        spool = ctx.enter_context(tc.tile_pool(name="sq", bufs=cfg['spool_bufs']))
        upool = ctx.enter_context(tc.tile_pool(name="upsum", bufs=2, space="PSUM"))
        apool = ctx.enter_context(tc.tile_pool(name="ampsum", bufs=2, space="PSUM"))
        mpool = ctx.enter_context(tc.tile_pool(name="smpsum", bufs=1, space="PSUM"))

        VE, GE, AE = nc.vector, nc.gpsimd, nc.scalar

        # ---- constants ----
        w3_sb = cpool.tile([P, 16 * K_MIX], F16)
        nc.sync.dma_start(w3_sb[:], w3d)
        e2_sb = cpool.tile([P, NB * TA], F16)
        nc.sync.dma_start(e2_sb[:], e2d)
        sstr_sb = cpool.tile([P, NB * S_PER_CORE], F16)
        nc.sync.dma_start(sstr_sb[:], sstrd)
        ones_sb = cpool.tile([P, 1], F16)
        nc.sync.dma_start(ones_sb[:], onesd)
        e_acc = cpool.tile([S_PER_CORE, 1], F32)
        nc.sync.dma_start(e_acc[:], einitd)
        bias_hpi = cpool.tile([P, 1], F32)
        nc.gpsimd.memset(bias_hpi[:], float(np.pi / 2))

        # ---- full-size gather tables / per-edge outputs ----
        g6 = gpool.tile([P, NT, 6], F32)
        goh = gpool.tile([P, NT, 4], F16)
        mmx = gpool.tile([P, NT, ASPAN * 16], F16)
        sh = gpool.tile([P, NT, 16], F16)
        pf = gpool.tile([P, NT, 128], F16)
        nc.gpsimd.memset(sh[:, :, M_OFF[0]:M_OFF[0] + 1], 1.0)
        # one-time zero of pf (pad cols must stay 0); body (repeat) excludes it
        for z0 in range(0, NT, TPB * 4):
            z1 = min(z0 + TPB * 4, NT)
            nc.sync.dma_start(
                pf[:, z0:z1, :],
                zd.rearrange("p (t c) -> p t c", c=128)[:, z0:z1, :])

        if cfg.get('chunk_sizes'):
            chunks = []
            b0 = 0
            for s in cfg['chunk_sizes']:
                chunks.append((b0, min(b0 + s, NB)))
                b0 += s
            chunks = [(a, b_) for (a, b_) in chunks if a < NB]
        elif cfg['nchunks'] == 'grow':
            chunks = _grow_chunks(NB)
        else:
            chunks = _chunks(NB, cfg['nchunks'])
            if cfg.get('first_small'):
                fs = cfg['first_small']
                flat = []
                for (a, b_) in chunks:
                    flat.extend(range(a, b_))
                sizes = [fs]
                rest = NB - fs
                n = cfg['nchunks'] - 1
                bs, rm = divmod(rest, n)
                sizes += [bs + (1 if i < rm else 0) for i in range(n)]
                chunks = []
                b0 = 0
                for s in sizes:
                    chunks.append((b0, b0 + s))
                    b0 += s

        for _rep in range(repeat):
          for (d0, d1) in _chunks(NB, cfg['ndma']):
            u0, u1 = d0 * TPB, d1 * TPB
            nc.sync.dma_start(
                g6[:, u0:u1, :],
                g6d.rearrange("p (t c) -> p t c", c=6)[:, u0:u1, :])
            nc.sync.dma_start(
                goh[:, u0:u1, :],
                gohd.rearrange("p (t c) -> p t c", c=4)[:, u0:u1, :])
            nc.sync.dma_start(
                mmx[:, u0:u1, :],
                mmxd.rearrange("p (t c) -> p t c", c=ASPAN * 16)[:, u0:u1, :])
          for (b0, b1) in chunks:
            t0, t1 = b0 * TPB, b1 * TPB
            T = t1 - t0

            def TT(eng, out, a, b_, op1, s=1.0, op0=OP.mult):
                if eng is GE:
                    # HW GPSIMD has no scalar_tensor_tensor opcode
                    assert s == 1.0 and op0 == OP.mult
                    eng.tensor_tensor(out, a, b_, op1)
                else:
                    eng.scalar_tensor_tensor(out, a, s, b_, op0, op1)

            gs = g6[:, t0:t1, 0:3]
            gr = g6[:, t0:t1, 3:6]
            rvec = tpool.tile([P, T, 3], F32, tag="rvec")
            TT(VE, rvec[:], gr, gs, OP.subtract)
            sq3 = tpool.tile([P, T, 3], F32, tag="sq3")
            e_sq3 = GE if 'sq3' in cfg['geo_pool'] else VE
            TT(e_sq3, sq3[:], rvec[:], rvec[:], OP.mult)
            r2 = tpool.tile([P, T, 1], F32, tag="r2")
            TT(VE, r2[:], sq3[:, :, 0:1], sq3[:, :, 1:2], OP.add)
            TT(VE, r2[:], sq3[:, :, 2:3], r2[:], OP.add, s=1e-12, op0=OP.add)
            # Newton fast-inverse-sqrt (2 iterations)
            ri = tpool.tile([P, T, 1], I32, tag="ri")
            VE.tensor_scalar(ri[:], r2[:].bitcast(I32), 1, None,
                             OP.logical_shift_right)
            VE.tensor_scalar(ri[:], ri[:], -1, MAGIC, OP.mult, OP.add)
            rinv = ri[:].bitcast(F32)
            h_t = tpool.tile([P, T, 1], F32, tag="h_t")
            w_t = tpool.tile([P, T, 1], F32, tag="w_t")
            NE = GE if 'newton' in cfg['geo_pool'] else VE
            for _it in range(2):
                # y' = y*(1.5 - 0.5*r2*y*y), 3 fused instrs
                TT(NE, h_t[:], rinv, rinv, OP.mult)
                TT(NE, w_t[:], h_t[:], r2[:], OP.mult, s=-0.5)
                TT(NE, rinv, w_t[:], rinv, OP.mult, s=1.5, op0=OP.add)
            # xr = min(r2*rinv/CUTOFF, 1); xrp = xr + 1e-3
            xr = tpool.tile([P, T, 1], F32, tag="xr")
            TT(VE, xr[:], r2[:], rinv, OP.mult, s=1.0 / CUTOFF)
            VE.tensor_scalar(xr[:], xr[:], 1.0, None, OP.min)
            xrp = tpool.tile([P, T, 1], F32, tag="xrp")
            VE.tensor_scalar(xrp[:], xr[:], 1e-3, None, OP.add)
            xrinv = tpool.tile([P, T, 1], F32, tag="xrinv")
            VE.reciprocal(xrinv[:], xrp[:])
            # u = rvec * rinv
            u = tpool.tile([P, T, 3], F32, tag="u")
            TT(VE, u[:], rvec[:], rinv.broadcast_to([P, T, 3]), OP.mult)
            fc = tpool.tile([P, T, 1], F32, tag="fc")
            AE.activation(fc[:], xr[:], ACTF.Sin,
                          bias=bias_hpi[:], scale=float(-np.pi))
            sin_t = tpool.tile([P, T, 8], F32, tag="sin_t")
            AE.activation(sin_t[:, :, 0:1], xr[:], ACTF.Sin,
                          scale=float(np.pi))
            stmp = tpool.tile([P, T, 1], F32, tag="stmp")
            CE = GE if 'cheb' in cfg['geo_pool'] else VE
            TT(CE, sin_t[:, :, 1:2], fc[:], sin_t[:, :, 0:1], OP.mult, s=2.0)
            for n in range(3, 9):
                TT(CE, stmp[:], fc[:], sin_t[:, :, n - 2:n - 1], OP.mult, s=2.0)
                TT(CE, sin_t[:, :, n - 1:n], stmp[:], sin_t[:, :, n - 3:n - 2],
                   OP.subtract, s=1.0, op0=OP.mult)
            # wfac = (fc+1)*xrinv ; ohw = goh*wfac
            wfac = tpool.tile([P, T, 1], F32, tag="wfac")
            TT(VE, wfac[:], fc[:], xrinv[:], OP.mult, s=1.0, op0=OP.add)
            ohw = tpool.tile([P, T, 4], F16, tag="ohw")
            TT(VE, ohw[:], goh[:, t0:t1, :], wfac[:].broadcast_to([P, T, 4]),
               OP.mult)
            # pf bands: tile t owns cols [32*(t%4), +32); 4D TT per group
            pfv = pf[:, t0:t1, :].rearrange("p (tq gg) c -> p tq gg c", gg=4)
            ohv = ohw[:].rearrange("p (tq gg) s -> p tq gg s", gg=4)
            siv = sin_t[:].rearrange("p (tq gg) r -> p tq gg r", gg=4)
            TQ = T // 4
            for g_ in range(4):
                e_pf = GE if g_ < cfg.get('pf_pool_n', 2) else VE
                e_pf.tensor_tensor(
                    pfv[:, :, g_, 32 * g_:32 * (g_ + 1)].rearrange(
                        "p tq (s r) -> p tq s r", r=8),
                    ohv[:, :, g_, :].unsqueeze(3).broadcast_to([P, TQ, 4, 8]),
                    siv[:, :, g_, :].unsqueeze(2).broadcast_to([P, TQ, 4, 8]),
                    OP.mult)

            # ---- spherical harmonics ----
            x = u[:, :, 0:1]
            y = u[:, :, 1:2]
            z = u[:, :, 2:3]
            shc = sh[:, t0:t1, :]
            o1, o2 = M_OFF[1], M_OFF[2]
            VE.tensor_scalar(shc[:, :, o1:o1 + 2], u[:, :, 1:3], C1, None, OP.mult)
            VE.tensor_scalar(shc[:, :, o1 + 2:o1 + 3], x, C1, None, OP.mult)
            u2 = tpool.tile([P, T, 3], F32, tag="u2")
            e_u2 = GE if 'u2' in cfg['geo_pool'] else VE
            TT(e_u2, u2[:], u[:], u[:], OP.mult)
            x2 = u2[:, :, 0:1]
            y2 = u2[:, :, 1:2]
            z2 = u2[:, :, 2:3]
            e_pr = GE if 'prods' in cfg['geo_pool'] else VE
            prods = tpool.tile([P, T, 3], F32, tag="prods")  # xy, yz, xz
            TT(e_pr, prods[:, :, 0:2], u[:, :, 0:2], u[:, :, 1:3], OP.mult)
            TT(e_pr, prods[:, :, 2:3], x, z, OP.mult)
            xy = prods[:, :, 0:1]
            yz = prods[:, :, 1:2]
            xz = prods[:, :, 2:3]
            VE.tensor_scalar(shc[:, :, o2:o2 + 2], prods[:, :, 0:2], C2A, None, OP.mult)
            VE.tensor_scalar(shc[:, :, o2 + 2:o2 + 3], z2, 3.0 * C2B, C2B,
                             OP.mult, OP.subtract)
            VE.tensor_scalar(shc[:, :, o2 + 3:o2 + 4], xz, C2A, None, OP.mult)
            xmy = tpool.tile([P, T, 1], F32, tag="xmy")
            TT(e_pr, xmy[:], x2, y2, OP.subtract)
            VE.tensor_scalar(shc[:, :, o2 + 4:o2 + 5], xmy[:], C2C, None, OP.mult)
            # l3
            LE = GE if 'l3' in cfg['geo_pool'] else VE
            t3a = tpool.tile([P, T, 1], F32, tag="t3a")
            TT(LE, t3a[:], x2, y2, OP.subtract, s=3.0)          # 3x2-y2
            TT(LE, shc[:, :, 0:1], t3a[:], y, OP.mult, s=C3A)
            TT(LE, shc[:, :, 1:2], xy, z, OP.mult, s=C3B)
            t511 = tpool.tile([P, T, 1], F32, tag="t511")
            LE.tensor_scalar(t511[:], z2, 5.0 * C3C, C3C, OP.mult, OP.subtract)
            TT(LE, shc[:, :, 2:3], t511[:], y, OP.mult)
            t533 = tpool.tile([P, T, 1], F32, tag="t533")
            LE.tensor_scalar(t533[:], z2, 5.0 * C3D, 3.0 * C3D, OP.mult, OP.subtract)
            TT(LE, shc[:, :, 3:4], t533[:], z, OP.mult)
            TT(LE, shc[:, :, 4:5], t511[:], x, OP.mult)
            TT(LE, shc[:, :, 5:6], xmy[:], z, OP.mult, s=C3B2)
            t3b = tpool.tile([P, T, 1], F32, tag="t3b")
            TT(LE, t3b[:], y2, x2, OP.subtract, s=3.0)   # 3y2 - x2
            TT(LE, shc[:, :, 6:7], t3b[:], x, OP.mult, s=-C3A)

            # ---- per-block phase 1 + phase 2 ----
            for b in range(b0, b1):
                sh_exp = epool.tile([P, TPB, ASPAN, 16], F16, tag="shexp")
                e_se = GE if (cfg['shexp_pool_mod'] and
                              b % cfg['shexp_pool_mod'] == 0) else VE
                e_se.tensor_tensor(
                   sh_exp[:],
                   sh[:, b * TPB:(b + 1) * TPB, :].unsqueeze(2)
                     .broadcast_to([P, TPB, ASPAN, 16]),
                   mmx[:, b * TPB:(b + 1) * TPB, :].rearrange(
                       "p t (a m) -> p t a m", m=16),
                   OP.mult)
                u_ps = upool.tile([P, CPG, ASPAN * 16], F32, tag="ups",
                                  space="PSUM")
                for cc_ in range(CPG):
                    for g in range(NGRP):
                        tl = cc_ * NGRP + g
                        nc.tensor.matmul(
                            u_ps[:, cc_, :],
                            lhsT=pf[:, b * TPB + tl, :],
                            rhs=sh_exp[:, tl, :, :].rearrange(
                                "p a m -> p (a m)"),
                            start=(g == 0), stop=(g == NGRP - 1))
                u_sb = epool.tile([P, CPG * ASPAN * 16], F16, tag="usb")
                AE.copy(u_sb[:], u_ps[:].rearrange("p c am -> p (c am)"))

                uv = u_sb[:].rearrange("q (ca m) -> q ca m", m=16)
                sq = spool.tile([P, TA, 16], F16, tag="sq")
                CA = CPG * ASPAN  # 30
                t4ab = spool.tile([P, 2, TA, 4], F16, tag="t4ab")
                for pair_i, (la, lb) in enumerate(((3, 0), (1, 2))):
                    am = apool.tile([P, NGRP, 256], F32, tag="am",
                                    space="PSUM")
                    amoff = 0
                    for l in (la, lb):
                        ml = M_LEN[l]
                        for g in range(NGRP):
                            nc.tensor.matmul(
                                am[:, g, amoff:amoff + CA * ml],
                                lhsT=w3_sb[:, (l * NGRP + g) * K_MIX:
                                           (l * NGRP + g + 1) * K_MIX],
                                rhs=uv[:, :, M_OFF[l]:M_OFF[l] + ml],
                                start=True, stop=True)
                        # squares: all 4 g at once -> sq[(g,c,a), moff:+ml]
                        # (3D APs; same element order as the 4D view)
                        dst = sq[:, :, M_OFF[l]:M_OFF[l] + ml]
                        src = am[:, :, amoff:amoff + CA * ml]
                        if f'l{l}' in cfg['sq_dve']:
                            VE.tensor_tensor(dst, src, src, OP.mult)
                        else:
                            AE.activation(dst, src, ACTF.Square)
                        amoff += CA * ml
                    # partial reduce of this pair's 8 m-cols: 8 -> 4
                    mo = pair_i * 8
                    VE.tensor_tensor(t4ab[:, pair_i, :, :],
                                     sq[:, :, mo:mo + 4],
                                     sq[:, :, mo + 4:mo + 8], OP.add)
                # combine pairs: 4+4 -> 4 -> 2 -> 1 (fp16 TT, 2x)
                t4 = spool.tile([P, TA, 4], F16, tag="t4")
                VE.tensor_tensor(t4[:], t4ab[:, 0, :, :], t4ab[:, 1, :, :],
                                 OP.add)
                t2 = spool.tile([P, TA, 2], F16, tag="t2")
                VE.tensor_tensor(t2[:], t4[:, :, 0:2], t4[:, :, 2:4], OP.add)
                Bt = spool.tile([P, TA], F16, tag="B")
                VE.tensor_tensor(Bt[:].unsqueeze(2), t2[:, :, 0:1],
                                 t2[:, :, 1:2], OP.add)
                e_b4 = VE if cfg['b4h_eng'] == 'dve' else GE
                B4 = spool.tile([P, TA], F16, tag="B4")
                e_b4.tensor_tensor(B4[:], Bt[:], Bt[:], OP.mult)
                H = spool.tile([P, TA], F16, tag="H")
                e_b4.tensor_tensor(H[:], B4[:], e2_sb[:, b * TA:(b + 1) * TA],
                                   OP.mult)
                at_ps = mpool.tile([TA, 1], F32, tag="at", space="PSUM")
                nc.tensor.matmul(at_ps[:], lhsT=H[:], rhs=ones_sb[:],
                                 start=True, stop=True)
                at_sb = spool.tile([TA, 1], F16, tag="atsb")
                if cfg.get('atsb_dve'):
                    VE.tensor_copy(at_sb[:], at_ps[:])
                else:
                    AE.copy(at_sb[:], at_ps[:])
                eb_ps = mpool.tile([S_PER_CORE, 1], F32, tag="eb", space="PSUM")
                nc.tensor.matmul(
                    eb_ps[:],
                    lhsT=sstr_sb[0:TA, b * S_PER_CORE:(b + 1) * S_PER_CORE],
                    rhs=at_sb[:], start=True, stop=True)
                VE.scalar_tensor_tensor(e_acc[:], e_acc[:], 1.0, eb_ps[:],
                                        OP.mult, OP.add)

        nc.sync.dma_start(eout, e_acc[:])

    nc.compile()
    return nc


_CACHE = {}


def _get_program(NT, NB):
    key = (NT, NB)
    if key not in _CACHE:
        _CACHE[key] = build_program(NT, NB)
    return _CACHE[key]


def run(inputs, trace=False, **kwargs):
    in_maps, NT, NB = _preprocess(inputs)
    nc = _get_program(NT, NB)
    res = run_bass_kernel_spmd(nc, in_maps, core_ids=list(range(N_CORES)),
                               trace=trace, **kwargs)
    out = np.concatenate([res.results[c]['eout'][:, 0] for c in range(N_CORES)])
    return out.astype(np.float32), res


def kernel(**inputs):
    out, _ = run(inputs)
    return out
